# revision 23
# baseline (speedup 1.0000x reference)
"""BiDAF attention kernel for Trainium2 (8 NeuronCores, data-parallel over batch).

Problem (per full input): B=16, L=M=1024, H=128
  s  = text@tw + (mod@mw).T + (text*tmw)@mod.T + bias          (B, L, M)
  p1 = softmax_M(mmask*s + (1-mmask)*NEG)
  p2 = softmax_L(tmask*s + (1-tmask)*NEG)
  a  = p1 @ mod
  b  = p1 @ p2.T @ text        (computed as p1 @ (p2.T @ text))
  out = [text, a, text*a, text*b]                               (B, L, 4H)

Decomposition (device does ONLY matmuls + plain exp + small normalization):
  * p1num[l,m] = exp(s2[l,m]) * g[m],  g = exp(s1 + (mmask-1)*3e4)   (s0, bias drop)
  * p2num[l,m] = exp(s2[l,m]) * h[l],  h = exp(s0 + (tmask-1)*3e4)   (s1, bias drop)
  * g folds into the final-matmul rhs rows ([g*mod | g | g*wq]); h folds into the
    q2 rhs rows ([h*textg | h]): both device activations are a PLAIN exp.
  * masked m / l rows are host-compacted (gather to MU/LU chunks of 128);
    padding rows carry g=0 / h=0 so they contribute exactly 0.
  * all matmul operands bf16 (host-pretransposed); PSUM f32; device never
    transposes or casts.
  * input tensors are packed so each batch needs few large DMAs, split across
    the sync HWDGE ring and the gpsimd SWDGE ring; outputs stream back on both.
  * PE + ACT-table warmup runs during the initial DMA wait.
  * q2 k-chunks and final l-chunks are processed in PAIRS sharing one PSUM
    tile, so the normalization/assembly DVE ops are batched (fewer, larger).

Each of the 8 cores processes 2 batch items; no cross-core communication.
"""

import numpy as np

B, L, M, H = 16, 1024, 1024, 128
NCORES = 8
BPC = B // NCORES  # batches per core
P = 128
LT = L // P
NEGB = 30000.0

_CACHE = {}


def _build(MU, LU):
    """Per-core Bass program for MU gathered m-chunks and LU gathered l-chunks
    (SPMD: same NEFF on all 8 cores)."""
    from contextlib import ExitStack

    import concourse.bass as bass
    import concourse.mybir as mybir
    import concourse.tile as tile
    from concourse import bacc
    from concourse.bass import ts

    f32 = mybir.dt.float32
    bf16 = mybir.dt.bfloat16
    f8 = mybir.dt.float8e4
    Exp = mybir.ActivationFunctionType.Exp
    DR = mybir.MatmulPerfMode.DoubleRow

    MG, LG = MU * P, LU * P
    NA = MG + LG            # packA cols: [modTg | xgT]
    NS = LU * (H + 1)       # stxtg pack width
    NWQ = 272               # modwq row: [g*mod(128) | g | g*wq(128) | pad] %16==0

    nc = bacc.Bacc(name="bidaf8")
    packA_d = nc.dram_tensor("packA", (BPC, P, NA), bf16, kind="ExternalInput").ap()
    txtT_d = nc.dram_tensor("txtT", (BPC, P, L), bf16, kind="ExternalInput").ap()
    stxtg_d = nc.dram_tensor("stxtg", (BPC, P, NS), bf16, kind="ExternalInput").ap()
    modc_d = nc.dram_tensor("modc", (BPC, P, MU, NWQ), f8, kind="ExternalInput").ap()
    txt_d = nc.dram_tensor("txt", (BPC, P, LT, H), f32, kind="ExternalInput").ap()
    out_d = nc.dram_tensor("out", (BPC, L, 4 * H), f32, kind="ExternalOutput").ap()
    warm_d = nc.dram_tensor("warm", (P, 8), f32, kind="ExternalOutput").ap()

    def oview(b):
        # (P, LT, 4H): partition p, chunk j <-> output row l = j*128 + p
        return out_d[b].rearrange("(o p) c -> p o c", p=P)

    with tile.TileContext(nc) as tc, ExitStack() as ctx:
        const = ctx.enter_context(tc.tile_pool(name="const", bufs=1))
        inp = ctx.enter_context(tc.tile_pool(name="inp", bufs=2))
        sc = ctx.enter_context(tc.tile_pool(name="sc", bufs=2))
        outp = ctx.enter_context(tc.tile_pool(name="outp", bufs=4))
        small = ctx.enter_context(tc.tile_pool(name="small", bufs=4))
        ps_sc = ctx.enter_context(tc.tile_pool(name="ps_sc", bufs=2, space="PSUM"))
        ps_q = ctx.enter_context(tc.tile_pool(name="ps_q", bufs=2, space="PSUM"))

        # ---- warmup: keep PE busy + preload the exp ACT table while the first
        # input DMAs are in flight (both outputs feed warm_d so nothing is DCEd)
        wsrc = const.tile([P, 512], bf16)
        nc.vector.memset(wsrc, 0.0)
        wps = ps_q.tile([P, 2, 512], f32, tag="f")
        NWARM = 10
        for i in range(NWARM):
            nc.tensor.matmul(wps[:, 0, :], wsrc[:, 0:P], wsrc,
                             start=(i == 0), stop=(i == NWARM - 1))
        warmsb = const.tile([P, 8], f32)
        nc.vector.tensor_copy(warmsb[:, 0:4], wps[:, 0, 0:4])
        nc.scalar.activation(warmsb[:, 4:8], wps[:, 0, 4:8], Exp)
        e1bias = const.tile([P, 1], f32)
        nc.vector.memset(e1bias, -1.1)

        st = []
        for b in range(BPC):
            d = {}
            # [modTg | xgT] -- needed first (E2/E1T matmuls); sync HWDGE ring
            d["packA"] = inp.tile([P, NA], bf16, tag="packA", name="packA")
            nc.sync.dma_start(d["packA"], packA_d[b])
            # txtT + [stxtg | modc(full width, wq slot junk)] -- scalar HWDGE
            # ring so they do not serialize behind the packA loads
            d["txtT"] = inp.tile([P, L], bf16, tag="txtT", name="txtT")
            nc.scalar.dma_start(d["txtT"], txtT_d[b])
            d["stxtg_t"] = inp.tile([P, NS], bf16, tag="stxtg_t", name="stxtg_t")
            nc.scalar.dma_start(d["stxtg_t"], stxtg_d[b])
            d["modwq"] = inp.tile([P, MU, NWQ], f8, tag="modwq", name="modwq")
            nc.scalar.dma_start(d["modwq"], modc_d[b])
            # text rows f32 (output col0 + products); gpsimd SWDGE ring
            d["txt"] = inp.tile([P, LT, H], f32, tag="txt", name="txt")
            nc.gpsimd.dma_start(d["txt"], txt_d[b])
            nc.gpsimd.dma_start(oview(b)[:, :, 0:H], d["txt"])
            d["modTg"] = d["packA"][:, 0:MG]
            d["xgT"] = d["packA"][:, MG:NA]
            d["stxtg"] = d["stxtg_t"].rearrange("p (c n) -> p c n", n=H + 1)
            st.append(d)

        def do_E2(b):
            d = st[b]
            # E2[lg, mg] = exp(s2g): p2 numerator / h (gathered l x gathered m)
            E2 = sc.tile([P, LU, MG], bf16, tag="E2", name="E2")
            for c in range(LU):
                sp = ps_sc.tile([P, MG], f32, tag="s", name="sp")
                for i in range(0, MG, 512):
                    n = min(512, MG - i)
                    nc.tensor.matmul(sp[:, i : i + n], d["xgT"][:, ts(c, P)],
                                     d["modTg"][:, i : i + n],
                                     start=True, stop=True)
                nc.scalar.activation(E2[:, c, :], sp, Exp)
            d["E2"] = E2

        def do_E1T_chunk(b, k):
            d = st[b]
            if k % 2 == 0:
                # E1T[mg, l] = exp(s2.T - 1.1): p1 numerator / g; fp8 output,
                # the shift keeps exp under the e4m3 max and cancels in the
                # a/b normalization by D1. One tile per k-pair so the final
                # DoubleRow matmuls start as soon as their pair is ready.
                d["E1Tp%d" % (k // 2)] = sc.tile(
                    [P, 2, L], f8, tag="E1Tp%d" % (k // 2),
                    name="E1Tp%d" % (k // 2))
            sp = ps_sc.tile([P, L], f32, tag="s", name="sp")
            for i in range(0, L, 512):
                nc.tensor.matmul(sp[:, i : i + 512], d["modTg"][:, ts(k, P)],
                                 d["txtT"][:, i : i + 512],
                                 start=True, stop=True)
            nc.scalar.activation(d["E1Tp%d" % (k // 2)][:, k % 2, :], sp, Exp,
                                 bias=e1bias)

        def do_q2_pair(b, kk):
            # q2[mg] = sum_lg E2[lg,mg] * [h*textg | h][lg] = [p2T@text*D2 | D2]
            d = st[b]
            E2, stxtg, modwq = d["E2"], d["stxtg"], d["modwq"]
            nk = min(2, MU - 2 * kk)
            qp = ps_q.tile([P, 2, 256], f32, tag="f", name="qp")
            for q in range(nk):
                k = 2 * kk + q
                for c in range(LU):
                    nc.tensor.matmul(qp[:, q, 0 : H + 1],
                                     E2[:, c, ts(k, P)], stxtg[:, c, :],
                                     start=(c == 0), stop=(c == LU - 1))
            recs = small.tile([P, 2], f32, tag="recs", name="recs")
            nc.vector.reciprocal(recs[:, 0:nk], qp[:, 0:nk, H])
            grecs = small.tile([P, 2], f32, tag="grecs", name="grecs")
            nc.vector.tensor_mul(grecs[:, 0:nk], recs[:, 0:nk],
                                 modwq[:, 2 * kk : 2 * kk + nk, H])
            nc.vector.tensor_mul(
                modwq[:, 2 * kk : 2 * kk + nk, H + 1 : 2 * H + 1],
                qp[:, 0:nk, 0:H],
                grecs[:, 0:nk, None].to_broadcast((P, nk, H)))

        def do_final_pair(b, jj):
            # [a_raw | D1 | b_raw](l) = sum_mg E1T[mg,l]*[g*mod | g | g*wq][mg]
            d = st[b]
            modwq, txt = d["modwq"], d["txt"]
            fp = ps_q.tile([P, 2, 512], f32, tag="f", name="fp")
            for q in range(2):
                jsl = ts(2 * jj + q, P)
                for kk in range(MU // 2):
                    nc.tensor.matmul(fp[:, q, 0:NWQ],
                                     d["E1Tp%d" % kk][:, :, jsl],
                                     modwq[:, 2 * kk : 2 * kk + 2, :],
                                     start=(kk == 0), stop=False,
                                     perf_mode=DR)
                for k in range(2 * (MU // 2), MU):
                    nc.tensor.matmul(fp[:, q, 0:NWQ],
                                     d["E1Tp%d" % (k // 2)][:, k % 2, jsl],
                                     modwq[:, k, :],
                                     start=False, stop=(k == MU - 1))
            rec2 = small.tile([P, 2], f32, tag="rec2", name="rec2")
            nc.vector.reciprocal(rec2, fp[:, :, H])
            o = outp.tile([P, 2, 3 * H], f32, tag="o", name="o")
            # o = [a | text*a | text*b] per j; first write [a_n | b_n] into
            # cols {0:H, 2H:3H}, then multiply by text into {H:2H, 2H:3H}
            # (tb half is an aligned elementwise in-place multiply).
            ov = o.rearrange("p j (c h) -> p j c h", h=H)
            ab_raw = fp[:, :, 0 : 2 * H + 2].rearrange(
                "p j (c n) -> p j c n", n=H + 1)[:, :, :, 0:H]
            nc.vector.tensor_mul(
                ov[:, :, 0:3:2, :], ab_raw,
                rec2[:, :, None, None].to_broadcast((P, 2, 2, H)))
            peng = nc.gpsimd if jj % 2 == 0 else nc.vector
            peng.tensor_mul(
                ov[:, :, 1:3, :], ov[:, :, 0:3:2, :],
                txt[:, 2 * jj : 2 * jj + 2, None, :].to_broadcast((P, 2, 2, H)))
            eng = nc.gpsimd if jj % 2 == 0 else nc.sync
            eng.dma_start(oview(b)[:, 2 * jj : 2 * jj + 2, H:], o)

        # schedule: b0 scores; b1 E2; then b1's E1T chunks interleaved with
        # b0's q2 pairs (q2 fills PE slot-waits during the ACT-paced E1T
        # phase); b0 final; b1 q2; b1 final.
        do_E2(0)
        for k in range(MU):
            do_E1T_chunk(0, k)
        do_E2(1)
        for k in range(MU):
            do_E1T_chunk(1, k)
            if k < (MU + 1) // 2:
                do_q2_pair(0, k)
        for jj in range(LT // 2):
            do_final_pair(0, jj)
        for kk in range((MU + 1) // 2):
            do_q2_pair(1, kk)
        for jj in range(LT // 2):
            do_final_pair(1, jj)

        nc.sync.dma_start(warm_d, warmsb)
    nc.compile()
    return nc


def get_nc(MU, LU):
    key = (MU, LU)
    if key not in _CACHE:
        _CACHE[key] = _build(MU, LU)
    return _CACHE[key]


def make_in_maps(text, modality, text_mask, modality_mask,
                 text_weight, modality_weight, text_modality_weight):
    import ml_dtypes

    bf16 = ml_dtypes.bfloat16
    text = np.ascontiguousarray(np.asarray(text, dtype=np.float32))
    modality = np.ascontiguousarray(np.asarray(modality, dtype=np.float32))
    text_mask = np.asarray(text_mask).astype(np.int32)
    modality_mask = np.asarray(modality_mask).astype(np.int32)
    wt = np.asarray(text_weight, dtype=np.float32).reshape(H)
    wm = np.asarray(modality_weight, dtype=np.float32).reshape(H)
    wtm = np.asarray(text_modality_weight, dtype=np.float32).reshape(H)

    LU = max(1, int(-(-int(text_mask.sum(axis=1).max()) // P)))
    MU = max(1, int(-(-int(modality_mask.sum(axis=1).max()) // P)))
    LG, MG = LU * P, MU * P

    s0 = text @ wt                                   # (B, L)
    s1 = modality @ wm                               # (B, M)
    with np.errstate(under="ignore"):
        h = np.exp(s0 + (text_mask - 1.0) * NEGB).astype(np.float32)
        # e^-2 shift guards g*mod against the fp8e4m3 max (448); it cancels
        # exactly in the a/b normalization by D1
        g = np.exp(s1 - 2.0 + (modality_mask - 1.0) * NEGB).astype(np.float32)

    f8 = ml_dtypes.float8_e4m3fn
    NA = MG + LG
    NS = LU * (H + 1)
    NWQ = 272
    in_maps = []
    for c in range(NCORES):
        packA = np.zeros((BPC, P, NA), bf16)
        txtT = np.zeros((BPC, P, L), bf16)
        stxtg = np.zeros((BPC, P, NS), bf16)
        modc = np.zeros((BPC, P, MU, NWQ), f8)
        txt = np.empty((BPC, P, LT, H), np.float32)
        for b in range(BPC):
            gb = BPC * c + b
            tl = np.argsort(1 - text_mask[gb], kind="stable")[:LG]
            tm = np.argsort(1 - modality_mask[gb], kind="stable")[:MG]
            txtw = text[gb] * wtm                    # (L, H) scaled by tmw
            packA[b, :, 0:MG] = modality[gb][tm].T.astype(bf16)
            packA[b, :, MG:NA] = txtw[tl].T.astype(bf16)
            txtT[b] = txtw.T.astype(bf16)
            stx = np.concatenate(
                [text[gb][tl] * h[gb][tl, None], h[gb][tl, None]], axis=1)
            stxtg[b] = (stx.reshape(LU, P, H + 1).transpose(1, 0, 2)
                        .reshape(P, NS).astype(bf16))
            mdc = np.zeros((MG, NWQ), np.float32)
            mdc[:, 0:H] = modality[gb][tm] * g[gb][tm, None]
            mdc[:, H] = g[gb][tm]
            modc[b] = mdc.reshape(MU, P, NWQ).transpose(1, 0, 2).astype(f8)
            txt[b] = text[gb].reshape(LT, P, H).transpose(1, 0, 2)
        in_maps.append({"packA": np.ascontiguousarray(packA),
                        "txtT": np.ascontiguousarray(txtT),
                        "stxtg": np.ascontiguousarray(stxtg),
                        "modc": np.ascontiguousarray(modc),
                        "txt": np.ascontiguousarray(txt)})
    return in_maps, MU, LU


def kernel(text, modality, text_mask, modality_mask,
           text_weight, modality_weight, text_modality_weight, bias,
           trace=False):
    from concourse.bass_utils import run_bass_kernel_spmd

    in_maps, MU, LU = make_in_maps(text, modality, text_mask, modality_mask,
                                   text_weight, modality_weight,
                                   text_modality_weight)
    nc = get_nc(MU, LU)
    res = run_bass_kernel_spmd(nc, in_maps, core_ids=list(range(NCORES)),
                               trace=trace)
    outp = np.concatenate([r["out"] for r in res.results], axis=0)
    if trace:
        kernel.last_result = res
    return outp


# revision 24
# speedup vs baseline: 1.0040x; 1.0040x over previous
"""BiDAF attention kernel for Trainium2 (8 NeuronCores, data-parallel over batch).

Problem (per full input): B=16, L=M=1024, H=128
  s  = text@tw + (mod@mw).T + (text*tmw)@mod.T + bias          (B, L, M)
  p1 = softmax_M(mmask*s + (1-mmask)*NEG)
  p2 = softmax_L(tmask*s + (1-tmask)*NEG)
  a  = p1 @ mod
  b  = p1 @ p2.T @ text        (computed as p1 @ (p2.T @ text))
  out = [text, a, text*a, text*b]                               (B, L, 4H)

Decomposition (device does ONLY matmuls + plain exp + small normalization):
  * p1num[l,m] = exp(s2[l,m]) * g[m],  g = exp(s1 + (mmask-1)*3e4)   (s0, bias drop)
  * p2num[l,m] = exp(s2[l,m]) * h[l],  h = exp(s0 + (tmask-1)*3e4)   (s1, bias drop)
  * g folds into the final-matmul rhs rows ([g*mod | g | g*wq]); h folds into the
    q2 rhs rows ([h*textg | h]): both device activations are a PLAIN exp.
  * masked m / l rows are host-compacted (gather to MU/LU chunks of 128);
    padding rows carry g=0 / h=0 so they contribute exactly 0.
  * all matmul operands bf16 (host-pretransposed); PSUM f32; device never
    transposes or casts.
  * input tensors are packed so each batch needs few large DMAs, split across
    the sync HWDGE ring and the gpsimd SWDGE ring; outputs stream back on both.
  * PE + ACT-table warmup runs during the initial DMA wait.
  * q2 k-chunks and final l-chunks are processed in PAIRS sharing one PSUM
    tile, so the normalization/assembly DVE ops are batched (fewer, larger).

Each of the 8 cores processes 2 batch items; no cross-core communication.
"""

import numpy as np

B, L, M, H = 16, 1024, 1024, 128
NCORES = 8
BPC = B // NCORES  # batches per core
P = 128
LT = L // P
NEGB = 30000.0

_CACHE = {}


def _build(MU, LU):
    """Per-core Bass program for MU gathered m-chunks and LU gathered l-chunks
    (SPMD: same NEFF on all 8 cores)."""
    from contextlib import ExitStack

    import concourse.bass as bass
    import concourse.mybir as mybir
    import concourse.tile as tile
    from concourse import bacc
    from concourse.bass import ts

    f32 = mybir.dt.float32
    bf16 = mybir.dt.bfloat16
    f8 = mybir.dt.float8e4
    Exp = mybir.ActivationFunctionType.Exp
    DR = mybir.MatmulPerfMode.DoubleRow

    MG, LG = MU * P, LU * P
    NA = MG + LG            # packA cols: [modTg | xgT]
    NS = LU * (H + 1)       # stxtg pack width
    NWQ = 272               # modwq row: [g*mod(128) | g | g*wq(128) | pad] %16==0

    nc = bacc.Bacc(name="bidaf8")
    packA_d = nc.dram_tensor("packA", (BPC, P, NA), bf16, kind="ExternalInput").ap()
    txtT_d = nc.dram_tensor("txtT", (BPC, P, L), bf16, kind="ExternalInput").ap()
    stxtg_d = nc.dram_tensor("stxtg", (BPC, P, NS), bf16, kind="ExternalInput").ap()
    modc_d = nc.dram_tensor("modc", (BPC, P, MU, NWQ), f8, kind="ExternalInput").ap()
    txt_d = nc.dram_tensor("txt", (BPC, P, LT, H), f32, kind="ExternalInput").ap()
    out_d = nc.dram_tensor("out", (BPC, L, 4 * H), f32, kind="ExternalOutput").ap()
    warm_d = nc.dram_tensor("warm", (P, 8), f32, kind="ExternalOutput").ap()

    def oview(b):
        # (P, LT, 4H): partition p, chunk j <-> output row l = j*128 + p
        return out_d[b].rearrange("(o p) c -> p o c", p=P)

    with tile.TileContext(nc) as tc, ExitStack() as ctx:
        const = ctx.enter_context(tc.tile_pool(name="const", bufs=1))
        inp = ctx.enter_context(tc.tile_pool(name="inp", bufs=2))
        sc = ctx.enter_context(tc.tile_pool(name="sc", bufs=2))
        outp = ctx.enter_context(tc.tile_pool(name="outp", bufs=4))
        small = ctx.enter_context(tc.tile_pool(name="small", bufs=4))
        ps_sc = ctx.enter_context(tc.tile_pool(name="ps_sc", bufs=2, space="PSUM"))
        ps_q = ctx.enter_context(tc.tile_pool(name="ps_q", bufs=2, space="PSUM"))

        # ---- warmup: keep PE busy + preload the exp ACT table while the first
        # input DMAs are in flight (both outputs feed warm_d so nothing is DCEd)
        wsrc = const.tile([P, 512], bf16)
        nc.vector.memset(wsrc, 0.0)
        wps = ps_q.tile([P, 2, 512], f32, tag="f")
        NWARM = 10
        for i in range(NWARM):
            nc.tensor.matmul(wps[:, 0, :], wsrc[:, 0:P], wsrc,
                             start=(i == 0), stop=(i == NWARM - 1))
        warmsb = const.tile([P, 8], f32)
        nc.vector.tensor_copy(warmsb[:, 0:4], wps[:, 0, 0:4])
        nc.scalar.activation(warmsb[:, 4:8], wps[:, 0, 4:8], Exp)
        e1bias = const.tile([P, 1], f32)
        nc.vector.memset(e1bias, -1.1)

        st = []
        for b in range(BPC):
            d = {}
            # [modTg | xgT] -- needed first (E2/E1T matmuls); sync HWDGE ring
            d["packA"] = inp.tile([P, NA], bf16, tag="packA", name="packA")
            nc.sync.dma_start(d["packA"], packA_d[b])
            # txtT + [stxtg | modc(full width, wq slot junk)] -- scalar HWDGE
            # ring so they do not serialize behind the packA loads
            d["txtT"] = inp.tile([P, L], bf16, tag="txtT", name="txtT")
            nc.scalar.dma_start(d["txtT"], txtT_d[b])
            d["stxtg_t"] = inp.tile([P, NS], bf16, tag="stxtg_t", name="stxtg_t")
            nc.scalar.dma_start(d["stxtg_t"], stxtg_d[b])
            d["modwq"] = inp.tile([P, MU, NWQ], f8, tag="modwq", name="modwq")
            nc.scalar.dma_start(d["modwq"], modc_d[b])
            # text rows f32 (output col0 + products); gpsimd SWDGE ring
            d["txt"] = inp.tile([P, LT, H], f32, tag="txt", name="txt")
            nc.gpsimd.dma_start(d["txt"], txt_d[b])
            nc.gpsimd.dma_start(oview(b)[:, :, 0:H], d["txt"])
            d["modTg"] = d["packA"][:, 0:MG]
            d["xgT"] = d["packA"][:, MG:NA]
            d["stxtg"] = d["stxtg_t"].rearrange("p (c n) -> p c n", n=H + 1)
            st.append(d)

        def do_E2(b):
            d = st[b]
            # E2[lg, mg] = exp(s2g): p2 numerator / h (gathered l x gathered m)
            E2 = sc.tile([P, LU, MG], bf16, tag="E2", name="E2")
            for c in range(LU):
                sp = ps_sc.tile([P, MG], f32, tag="s", name="sp")
                for i in range(0, MG, 512):
                    n = min(512, MG - i)
                    nc.tensor.matmul(sp[:, i : i + n], d["xgT"][:, ts(c, P)],
                                     d["modTg"][:, i : i + n],
                                     start=True, stop=True)
                nc.scalar.activation(E2[:, c, :], sp, Exp)
            d["E2"] = E2

        def do_E1T_chunk(b, k):
            d = st[b]
            if k == 0:
                # E1T[mg, l] = exp(s2.T - 1.1): p1 numerator / g; fp8 output,
                # the shift keeps exp under the e4m3 max and cancels in the
                # a/b normalization by D1
                d["E1T"] = sc.tile([P, MU, L], f8, tag="E1T", name="E1T")
            sp = ps_sc.tile([P, L], f32, tag="s", name="sp")
            for i in range(0, L, 512):
                nc.tensor.matmul(sp[:, i : i + 512], d["modTg"][:, ts(k, P)],
                                 d["txtT"][:, i : i + 512],
                                 start=True, stop=True)
            nc.scalar.activation(d["E1T"][:, k, :], sp, Exp, bias=e1bias)

        def do_q2_pair(b, kk):
            # q2[mg] = sum_lg E2[lg,mg] * [h*textg | h][lg] = [p2T@text*D2 | D2]
            d = st[b]
            E2, stxtg, modwq = d["E2"], d["stxtg"], d["modwq"]
            nk = min(2, MU - 2 * kk)
            qp = ps_q.tile([P, 2, 256], f32, tag="f", name="qp")
            for q in range(nk):
                k = 2 * kk + q
                for c in range(LU):
                    nc.tensor.matmul(qp[:, q, 0 : H + 1],
                                     E2[:, c, ts(k, P)], stxtg[:, c, :],
                                     start=(c == 0), stop=(c == LU - 1))
            recs = small.tile([P, 2], f32, tag="recs", name="recs")
            nc.vector.reciprocal(recs[:, 0:nk], qp[:, 0:nk, H])
            grecs = small.tile([P, 2], f32, tag="grecs", name="grecs")
            nc.vector.tensor_mul(grecs[:, 0:nk], recs[:, 0:nk],
                                 modwq[:, 2 * kk : 2 * kk + nk, H])
            nc.vector.tensor_mul(
                modwq[:, 2 * kk : 2 * kk + nk, H + 1 : 2 * H + 1],
                qp[:, 0:nk, 0:H],
                grecs[:, 0:nk, None].to_broadcast((P, nk, H)))

        def do_final_pair(b, jj):
            # [a_raw | D1 | b_raw](l) = sum_mg E1T[mg,l]*[g*mod | g | g*wq][mg]
            d = st[b]
            E1T, modwq, txt = d["E1T"], d["modwq"], d["txt"]
            fp = ps_q.tile([P, 2, 512], f32, tag="f", name="fp")
            for q in range(2):
                jsl = ts(2 * jj + q, P)
                for kk in range(MU // 2):
                    nc.tensor.matmul(fp[:, q, 0:NWQ],
                                     E1T[:, 2 * kk : 2 * kk + 2, jsl],
                                     modwq[:, 2 * kk : 2 * kk + 2, :],
                                     start=(kk == 0), stop=False,
                                     perf_mode=DR)
                for k in range(2 * (MU // 2), MU):
                    nc.tensor.matmul(fp[:, q, 0:NWQ],
                                     E1T[:, k, jsl], modwq[:, k, :],
                                     start=False, stop=(k == MU - 1))
            rec2 = small.tile([P, 2], f32, tag="rec2", name="rec2")
            nc.vector.reciprocal(rec2, fp[:, :, H])
            o = outp.tile([P, 2, 3 * H], f32, tag="o", name="o")
            # o = [a | text*a | text*b] per j; first write [a_n | b_n] into
            # cols {0:H, 2H:3H}, then multiply by text into {H:2H, 2H:3H}
            # (tb half is an aligned elementwise in-place multiply).
            ov = o.rearrange("p j (c h) -> p j c h", h=H)
            ab_raw = fp[:, :, 0 : 2 * H + 2].rearrange(
                "p j (c n) -> p j c n", n=H + 1)[:, :, :, 0:H]
            nc.vector.tensor_mul(
                ov[:, :, 0:3:2, :], ab_raw,
                rec2[:, :, None, None].to_broadcast((P, 2, 2, H)))
            peng = nc.gpsimd if jj % 2 == 0 else nc.vector
            peng.tensor_mul(
                ov[:, :, 1:3, :], ov[:, :, 0:3:2, :],
                txt[:, 2 * jj : 2 * jj + 2, None, :].to_broadcast((P, 2, 2, H)))
            eng = nc.gpsimd if jj % 2 == 0 else nc.sync
            eng.dma_start(oview(b)[:, 2 * jj : 2 * jj + 2, H:], o)

        # schedule: b0 scores; b1 E2; then b1's E1T chunks interleaved with
        # b0's q2 pairs (q2 fills PE slot-waits during the ACT-paced E1T
        # phase); b0 final; b1 q2; b1 final.
        do_E2(0)
        for k in range(MU):
            do_E1T_chunk(0, k)
        do_E2(1)
        for k in range(MU):
            do_E1T_chunk(1, k)
            if k < (MU + 1) // 2:
                do_q2_pair(0, k)
        for jj in range(LT // 2):
            do_final_pair(0, jj)
        for kk in range((MU + 1) // 2):
            do_q2_pair(1, kk)
        for jj in range(LT // 2):
            do_final_pair(1, jj)

        nc.sync.dma_start(warm_d, warmsb)
    nc.compile()
    return nc


def get_nc(MU, LU):
    key = (MU, LU)
    if key not in _CACHE:
        _CACHE[key] = _build(MU, LU)
    return _CACHE[key]


def make_in_maps(text, modality, text_mask, modality_mask,
                 text_weight, modality_weight, text_modality_weight):
    import ml_dtypes

    bf16 = ml_dtypes.bfloat16
    text = np.ascontiguousarray(np.asarray(text, dtype=np.float32))
    modality = np.ascontiguousarray(np.asarray(modality, dtype=np.float32))
    text_mask = np.asarray(text_mask).astype(np.int32)
    modality_mask = np.asarray(modality_mask).astype(np.int32)
    wt = np.asarray(text_weight, dtype=np.float32).reshape(H)
    wm = np.asarray(modality_weight, dtype=np.float32).reshape(H)
    wtm = np.asarray(text_modality_weight, dtype=np.float32).reshape(H)

    LU = max(1, int(-(-int(text_mask.sum(axis=1).max()) // P)))
    MU = max(1, int(-(-int(modality_mask.sum(axis=1).max()) // P)))
    LG, MG = LU * P, MU * P

    s0 = text @ wt                                   # (B, L)
    s1 = modality @ wm                               # (B, M)
    with np.errstate(under="ignore"):
        h = np.exp(s0 + (text_mask - 1.0) * NEGB).astype(np.float32)
        # e^-2 shift guards g*mod against the fp8e4m3 max (448); it cancels
        # exactly in the a/b normalization by D1
        g = np.exp(s1 - 2.0 + (modality_mask - 1.0) * NEGB).astype(np.float32)

    f8 = ml_dtypes.float8_e4m3fn
    NA = MG + LG
    NS = LU * (H + 1)
    NWQ = 272
    in_maps = []
    for c in range(NCORES):
        packA = np.zeros((BPC, P, NA), bf16)
        txtT = np.zeros((BPC, P, L), bf16)
        stxtg = np.zeros((BPC, P, NS), bf16)
        modc = np.zeros((BPC, P, MU, NWQ), f8)
        txt = np.empty((BPC, P, LT, H), np.float32)
        for b in range(BPC):
            gb = BPC * c + b
            tl = np.argsort(1 - text_mask[gb], kind="stable")[:LG]
            tm = np.argsort(1 - modality_mask[gb], kind="stable")[:MG]
            txtw = text[gb] * wtm                    # (L, H) scaled by tmw
            packA[b, :, 0:MG] = modality[gb][tm].T.astype(bf16)
            packA[b, :, MG:NA] = txtw[tl].T.astype(bf16)
            txtT[b] = txtw.T.astype(bf16)
            stx = np.concatenate(
                [text[gb][tl] * h[gb][tl, None], h[gb][tl, None]], axis=1)
            stxtg[b] = (stx.reshape(LU, P, H + 1).transpose(1, 0, 2)
                        .reshape(P, NS).astype(bf16))
            mdc = np.zeros((MG, NWQ), np.float32)
            mdc[:, 0:H] = modality[gb][tm] * g[gb][tm, None]
            mdc[:, H] = g[gb][tm]
            modc[b] = mdc.reshape(MU, P, NWQ).transpose(1, 0, 2).astype(f8)
            txt[b] = text[gb].reshape(LT, P, H).transpose(1, 0, 2)
        in_maps.append({"packA": np.ascontiguousarray(packA),
                        "txtT": np.ascontiguousarray(txtT),
                        "stxtg": np.ascontiguousarray(stxtg),
                        "modc": np.ascontiguousarray(modc),
                        "txt": np.ascontiguousarray(txt)})
    return in_maps, MU, LU


def kernel(text, modality, text_mask, modality_mask,
           text_weight, modality_weight, text_modality_weight, bias,
           trace=False):
    from concourse.bass_utils import run_bass_kernel_spmd

    in_maps, MU, LU = make_in_maps(text, modality, text_mask, modality_mask,
                                   text_weight, modality_weight,
                                   text_modality_weight)
    nc = get_nc(MU, LU)
    res = run_bass_kernel_spmd(nc, in_maps, core_ids=list(range(NCORES)),
                               trace=trace)
    outp = np.concatenate([r["out"] for r in res.results], axis=0)
    if trace:
        kernel.last_result = res
    return outp
